# revision 18
# baseline (speedup 1.0000x reference)
"""Trainium2 Bass kernel for the pairwise-classifier loss.

Math: per branch, logits = x @ W + b with only 2 classes, so everything
reduces to the logit difference d = x . (W[:,1]-W[:,0]) + (b[1]-b[0]).
With x a concat of two gathered feature vectors, d splits into a sum of
two per-position projections:
    lo[b,n] = sum_c feats[b,c,n] * w[c]      (w = first 256 rows of dW)
    hi[b,n] = sum_c feats[b,c,n] * w[256+c]  (last 256 rows)
Per pair r: d_pos = lo[pb,pi] + hi[pb,pj] + db
            d_negA = lo[nb,ni] + hi[pb,pi] + db
            d_negB = lo[nb,ni] + hi[pb,pj] + db
and the double-softmax CE row loss, with y = tanh((d+db)/2)
(= 2*sigmoid(d+db)-1), E = exp(y), L = ln(E+1):
    loss_neg = L           (softplus(y))
    loss_pos = L - y       (softplus(-y))
The permutation inputs are irrelevant: the loss is a mean over rows.

Device plan (8 cores, pairs data-parallel 12500/core):
  1. Load features (f32->bf16 cast during DMA).
  2. PE: feats_block[128c,128n]^T @ Wp[128c,4] -> PSUM [128n, 4m],
     m = (row_lo,row_hi,col_lo,col_hi); 2 k-tiles accumulated; pack into
     PSUM banks; copy to SBUF; DMA to a DRAM table of 8-byte (lo,hi)
     pairs.
  3. Flat table offsets via DVE int ops; 3 indirect-DMA gathers of
     8-byte pairs per branch (12800 descriptors each).
  4. DVE adds -> ACT tanh/exp/ln with per-partition accum_out; pad
     partitions masked via the final dot vector; PE dot with 1/(3R)
     -> per-core partial; host sums the 8 partials.

Two program variants:
  - replicated (fallback): every core loads all 16MB of features and
    builds the whole table locally.  No cross-core communication.
  - sharded (default): core k receives only all_features[k] (2MB),
    computes its 4096-row chunk, and an 8-core AllGather assembles the
    full table on every core.
"""

import os

import numpy as np

import concourse.bass as bass
import concourse.bacc as bacc
import concourse.mybir as mybir
import concourse.tile as tile
from concourse.bass_utils import run_bass_kernel_spmd

F32 = mybir.dt.float32
BF16 = mybir.dt.bfloat16
I32 = mybir.dt.int32

B, C, N = 8, 256, 2048
R = 100000
NCORES = 8
PAIRS = R // NCORES          # 12500 pairs per core per branch
KW = 100                     # free-dim width of index tiles
P = 128
PADPAIRS = P * KW            # 12800
NVALID_P = PAIRS // KW       # 125 partitions hold real pairs
NSLOT = B * (N // P)         # 128 (b, nblock) slots
TROWS = P * NSLOT * 2        # 32768 8-byte (lo,hi) table rows


def _emit_weight_prep(nc, const, psmall, w_row, w_col, b_row, b_col):
    """Wp [128, 2kt*4m] bf16 projection weights and db128 [128, 2] f32
    holding (b[1]-b[0])/2 per branch broadcast to all partitions."""
    wr_raw = const.tile([P, 8], F32, tag="wr_raw")
    wc_raw = const.tile([P, 8], F32, tag="wc_raw")
    nc.sync.dma_start(
        out=wr_raw[:].rearrange("p (s t) -> p s t", s=4),
        in_=w_row[:].rearrange("(s p) t -> p s t", p=P),
    )
    nc.sync.dma_start(
        out=wc_raw[:].rearrange("p (s t) -> p s t", s=4),
        in_=w_col[:].rearrange("(s p) t -> p s t", p=P),
    )
    wdiff_r = const.tile([P, 4], F32, tag="wdiff_r")
    wdiff_c = const.tile([P, 4], F32, tag="wdiff_c")
    nc.vector.tensor_tensor(
        out=wdiff_r[:], in0=wr_raw[:, 1::2], in1=wr_raw[:, 0::2],
        op=mybir.AluOpType.subtract,
    )
    nc.vector.tensor_tensor(
        out=wdiff_c[:], in0=wc_raw[:, 1::2], in1=wc_raw[:, 0::2],
        op=mybir.AluOpType.subtract,
    )
    # Wp[:, kt*4 + m]: m = (row_lo, row_hi, col_lo, col_hi)
    wp = const.tile([P, 8], BF16, tag="wp")
    nc.vector.tensor_copy(out=wp[:, 0:8:4], in_=wdiff_r[:, 0:2])
    nc.vector.tensor_copy(out=wp[:, 1:8:4], in_=wdiff_r[:, 2:4])
    nc.vector.tensor_copy(out=wp[:, 2:8:4], in_=wdiff_c[:, 0:2])
    nc.vector.tensor_copy(out=wp[:, 3:8:4], in_=wdiff_c[:, 2:4])

    br_raw = const.tile([1, 2], F32, tag="br_raw")
    bc_raw = const.tile([1, 2], F32, tag="bc_raw")
    nc.sync.dma_start(out=br_raw[:], in_=b_row[:])
    nc.sync.dma_start(out=bc_raw[:], in_=b_col[:])
    db_rc = const.tile([1, 2], F32, tag="db_rc")
    nc.vector.tensor_tensor(
        out=db_rc[:, 0:1], in0=br_raw[:, 1:2], in1=br_raw[:, 0:1],
        op=mybir.AluOpType.subtract,
    )
    nc.vector.tensor_tensor(
        out=db_rc[:, 1:2], in0=bc_raw[:, 1:2], in1=bc_raw[:, 0:1],
        op=mybir.AluOpType.subtract,
    )
    # broadcast db/2 via a 0.5-valued ones row (tanh bias is db/2)
    half_row = const.tile([1, P], F32, tag="half_row")
    nc.vector.memset(half_row[:], 0.5)
    db_psum = psmall.tile([P, 2], F32, tag="db_psum")
    nc.tensor.matmul(
        db_psum[:], lhsT=half_row[:], rhs=db_rc[:], start=True, stop=True,
    )
    db128 = const.tile([P, 2], F32, tag="db128")
    nc.vector.tensor_copy(out=db128[:], in_=db_psum[:])
    return wp, db128


def _emit_offsets(nc, const, work, idx, qmul, bmul):
    """Load packed index lists and compute 8-byte-row table offsets
    off(b, n, br) = (n%128)*qmul + (n/128)*2 + b*bmul + br
    for e1=(pb,pi), e2=(pb,pj), e3=(nb,ni) per branch."""
    assert qmul & (qmul - 1) == 0
    qshift = qmul.bit_length() - 1
    idx_sb = const.tile([P, 10 * KW], I32, tag="idx_sb")
    nc.sync.dma_start(out=idx_sb[:], in_=idx[:])

    def off_tile(b_ap, n_ap, branch, name):
        t_lo = work.tile([P, KW], I32, tag=f"{name}_lo")
        t_hi = work.tile([P, KW], I32, tag=f"{name}_hi")
        t_b = work.tile([P, KW], I32, tag=f"{name}_b")
        out = const.tile([P, KW], I32, tag=f"{name}_out")
        # (n & 127) * qmul == (n & 127) << qshift; (n >> 7) * 2 ==
        # (n & ~127) >> 6 — keep each fused pair in one ALU class
        # (walrus rejects bitwise+arith mixes).
        nc.vector.tensor_scalar(
            out=t_lo[:], in0=n_ap, scalar1=127, scalar2=qshift,
            op0=mybir.AluOpType.bitwise_and,
            op1=mybir.AluOpType.logical_shift_left,
        )
        nc.vector.tensor_scalar(
            out=t_hi[:], in0=n_ap, scalar1=-128, scalar2=6,
            op0=mybir.AluOpType.bitwise_and,
            op1=mybir.AluOpType.logical_shift_right,
        )
        nc.vector.tensor_scalar(
            out=t_b[:], in0=b_ap, scalar1=bmul, scalar2=branch,
            op0=mybir.AluOpType.mult, op1=mybir.AluOpType.add,
        )
        nc.vector.tensor_tensor(
            out=t_lo[:], in0=t_lo[:], in1=t_hi[:], op=mybir.AluOpType.add,
        )
        nc.vector.tensor_tensor(
            out=out[:], in0=t_lo[:], in1=t_b[:], op=mybir.AluOpType.add,
        )
        return out

    def idx_list(branch, l):
        o = (branch * 5 + l) * KW
        return idx_sb[:, o:o + KW]

    offs = {}
    for br in (0, 1):
        pb, pi, pj, nb, ni = (idx_list(br, l) for l in range(5))
        offs[br, "e1"] = off_tile(pb, pi, br, f"b{br}e1")
        offs[br, "e2"] = off_tile(pb, pj, br, f"b{br}e2")
        offs[br, "e3"] = off_tile(nb, ni, br, f"b{br}e3")
    return idx_sb, offs


def _emit_gather_and_loss(nc, const, work, psmall, t_dram, offs, db128,
                          partial, dep):
    """Indirect gathers from the table + tanh/exp/ln row losses +
    masked partition reduction into the per-core partial output."""
    # The qPoolDynamic pseudo-DMA lowering only supports a single sync
    # wait per instruction, but each gather needs {offsets ready (DVE),
    # table ready (dep)}.  Absorb the table-ready wait into one Pool
    # join op; same-engine program order then covers it for the
    # gathers, leaving them at most their one DVE wait.
    join_t = const.tile([1, 1], I32, tag="join_t")
    join = nc.gpsimd.memset(join_t[:], 0)
    tile.add_dep_helper(
        join.ins, dep.ins, sync=True, reason="join waits for proj table",
    )
    gath = {}
    for br in (0, 1):
        for e in ("e1", "e2", "e3"):
            g_t = const.tile([P, KW, 2], F32, tag=f"g{br}{e}")
            ins = nc.gpsimd.indirect_dma_start(
                out=g_t[:],
                out_offset=None,
                in_=t_dram[:],
                in_offset=bass.IndirectOffsetOnAxis(ap=offs[br, e][:], axis=0),
            )
            tile.add_dep_helper(
                ins.ins, join.ins, sync=False, reason="gather after join",
            )
            tile.add_dep_helper(
                ins.ins, dep.ins, sync=False, reason="gather reads proj table",
            )
            gath[br, e] = g_t

    # Engine instructions may carry only ONE sync wait after codegen,
    # but a d-add reads two gathers that complete on different DMA
    # queue sems.  Absorb each gather's completion into a tiny DVE
    # probe copy (one wait each); the adds then run wait-free in DVE
    # program order.
    probe = const.tile([1, 12], F32, tag="probe")
    probes = []
    for i, (br, e) in enumerate(
        (b, e) for b in (0, 1) for e in ("e1", "e2", "e3")
    ):
        pr = nc.vector.tensor_copy(
            out=probe[0:1, i * 2:i * 2 + 2], in_=gath[br, e][0:1, 0:1, 0:2],
        )
        probes.append(pr)

    # acc slots 0..5: sum(L) per rowset; 6..7: sum(y) of pos rowsets.
    acc = const.tile([P, 8], F32, tag="acc")
    d_tiles = []
    for br in (0, 1):
        a_ap = gath[br, "e1"][:, :, 0:1]
        b_ap = gath[br, "e1"][:, :, 1:2]
        c_ap = gath[br, "e2"][:, :, 1:2]
        d_ap = gath[br, "e3"][:, :, 0:1]
        for nm, in0, in1 in (
            ("pos", a_ap, c_ap),
            ("negA", d_ap, b_ap),
            ("negB", d_ap, c_ap),
        ):
            d_t = work.tile([P, KW], F32, tag=f"d{br}{nm}")
            tt = nc.vector.tensor_tensor(
                out=d_t[:], in0=in0, in1=in1, op=mybir.AluOpType.add,
            )
            for pr in probes:
                tile.add_dep_helper(
                    tt.ins, pr.ins, sync=False, reason="adds after probes",
                )
            d_tiles.append((br, nm, d_t))

    bias_one = const.tile([P, 1], F32, tag="bias_one")
    nc.vector.memset(bias_one[:], 1.0)

    # phase 1: tanh (+ pos-row sum(y)) and exp — one ACT table set
    e_args = []
    for i, (br, nm, d_t) in enumerate(d_tiles):
        y_t = work.tile([P, KW], F32, tag=f"y{br}{nm}")
        kw = {}
        if nm == "pos":
            kw["accum_out"] = acc[:, 6 + br:7 + br]
        nc.scalar.activation(
            out=y_t[:], in_=d_t[:],
            func=mybir.ActivationFunctionType.Tanh,
            bias=db128[:, br:br + 1], scale=0.5, **kw,
        )
        e_t = work.tile([P, KW], F32, tag=f"e{br}{nm}")
        nc.scalar.activation(
            out=e_t[:], in_=y_t[:], func=mybir.ActivationFunctionType.Exp,
        )
        e_args.append((i, e_t))
    # phase 2: ln(E+1) with per-partition accumulation
    for i, e_t in e_args:
        l_t = work.tile([P, KW], F32, tag=f"l{i}")
        nc.scalar.activation(
            out=l_t[:], in_=e_t[:],
            func=mybir.ActivationFunctionType.Ln,
            bias=bias_one[:, 0:1], scale=1.0,
            accum_out=acc[:, i:i + 1],
        )

    # reduce: sum(L over 6 rowsets) - sum(y over pos rowsets)
    t_l = const.tile([P, 1], F32, tag="t_l")
    t_y = const.tile([P, 1], F32, tag="t_y")
    nc.vector.tensor_reduce(
        out=t_l[:], in_=acc[:, 0:6], axis=mybir.AxisListType.X,
        op=mybir.AluOpType.add,
    )
    nc.vector.tensor_reduce(
        out=t_y[:], in_=acc[:, 6:8], axis=mybir.AxisListType.X,
        op=mybir.AluOpType.add,
    )
    total = const.tile([P, 1], F32, tag="total")
    nc.vector.tensor_tensor(
        out=total[:], in0=t_l[:], in1=t_y[:], op=mybir.AluOpType.subtract,
    )
    # scale_vec[p] = 1/(3R) for p < NVALID_P else 0 (pad partitions)
    pidx = const.tile([P, 1], I32, tag="pidx")
    nc.gpsimd.iota(pidx[:], pattern=[[0, 1]], base=0, channel_multiplier=1)
    scale_vec = const.tile([P, 1], F32, tag="scale_vec")
    nc.vector.tensor_scalar(
        out=scale_vec[:], in0=pidx[:], scalar1=NVALID_P,
        scalar2=1.0 / (3.0 * R),
        op0=mybir.AluOpType.is_lt, op1=mybir.AluOpType.mult,
    )
    out_psum = psmall.tile([1, 1], F32, tag="out_psum")
    nc.tensor.matmul(
        out_psum[:], lhsT=total[:], rhs=scale_vec[:], start=True, stop=True,
    )
    out_sb = const.tile([1, 1], F32, tag="out_sb")
    nc.vector.tensor_copy(out=out_sb[:], in_=out_psum[:])
    nc.sync.dma_start(out=partial[:], in_=out_sb[0, :])


def _build_nc_replicated():
    """Every core loads all of all_features and builds the full table.
    Table row = (n%128)*256 + (b*16 + n/128)*2 + branch."""
    nc = bacc.Bacc()

    feats = nc.declare_dram_parameter("feats", [B, C, N], F32, isOutput=False)
    w_row = nc.declare_dram_parameter("w_row", [2 * C, 2], F32, isOutput=False)
    w_col = nc.declare_dram_parameter("w_col", [2 * C, 2], F32, isOutput=False)
    b_row = nc.declare_dram_parameter("b_row", [1, 2], F32, isOutput=False)
    b_col = nc.declare_dram_parameter("b_col", [1, 2], F32, isOutput=False)
    idx = nc.declare_dram_parameter("idx", [P, 10 * KW], I32, isOutput=False)
    partial = nc.declare_dram_parameter("partial", [1], F32, isOutput=True)

    t_dram = nc.dram_tensor("proj_table", [TROWS, 2], F32)

    with tile.TileContext(nc) as tc:
        with (
            tc.tile_pool(name="const", bufs=1) as const,
            tc.tile_pool(name="fbpool", bufs=8) as fbpool,
            tc.tile_pool(name="work", bufs=2) as work,
            tc.tile_pool(name="psum", bufs=2, space="PSUM") as psum,
            tc.tile_pool(name="psmall", bufs=1, space="PSUM") as psmall,
        ):
            wp, db128 = _emit_weight_prep(nc, const, psmall, w_row, w_col,
                                          b_row, b_col)
            _, offs = _emit_offsets(nc, const, work, idx, qmul=256, bmul=32)

            s_sb = const.tile([P, NSLOT * 4], F32, tag="s_sb")
            fb_tiles = []
            for b in range(B):
                fb = fbpool.tile([P, 2 * N], BF16, tag="fb")
                nc.gpsimd.dma_start(
                    out=fb[:].rearrange("p (kt n) -> p kt n", kt=2),
                    in_=feats[b].rearrange("(kt p) n -> p kt n", p=P),
                )
                fb_tiles.append(fb)

            for g in range(4):
                pt = psum.tile([P, P], F32, tag="pt")
                for s in range(32):
                    slot = g * 32 + s
                    b, blk = slot // 16, slot % 16
                    fb = fb_tiles[b]
                    nc.tensor.matmul(
                        pt[:, s * 4:(s + 1) * 4],
                        lhsT=fb[:, blk * P:(blk + 1) * P],
                        rhs=wp[:, 0:4], start=True, stop=False,
                    )
                    nc.tensor.matmul(
                        pt[:, s * 4:(s + 1) * 4],
                        lhsT=fb[:, N + blk * P:N + (blk + 1) * P],
                        rhs=wp[:, 4:8], start=False, stop=True,
                    )
                nc.vector.tensor_copy(out=s_sb[:, g * P:(g + 1) * P], in_=pt[:])

            t_write = nc.sync.dma_start(
                out=t_dram[:].rearrange("(q r) t -> q r t", q=P),
                in_=s_sb[:].rearrange("p (r t) -> p r t", t=2),
            )
            _emit_gather_and_loss(nc, const, work, psmall, t_dram, offs,
                                  db128, partial, dep=t_write)
    return nc


def _build_nc_sharded():
    """Core k receives only all_features[k] (feats_my [C, N]), computes
    its 4096-row chunk of the table, and an 8-core AllGather assembles
    the full table.  Table row = b*4096 + (n%128)*32 + (n/128)*2 + br."""
    nc = bacc.Bacc()

    feats = nc.declare_dram_parameter("feats_my", [C, N], F32, isOutput=False)
    w_row = nc.declare_dram_parameter("w_row", [2 * C, 2], F32, isOutput=False)
    w_col = nc.declare_dram_parameter("w_col", [2 * C, 2], F32, isOutput=False)
    b_row = nc.declare_dram_parameter("b_row", [1, 2], F32, isOutput=False)
    b_col = nc.declare_dram_parameter("b_col", [1, 2], F32, isOutput=False)
    idx = nc.declare_dram_parameter("idx", [P, 10 * KW], I32, isOutput=False)
    partial = nc.declare_dram_parameter("partial", [1], F32, isOutput=True)

    chunk_dram = nc.dram_tensor("proj_chunk", [TROWS // NCORES, 2], F32)
    t_dram = nc.dram_tensor("proj_table", [TROWS, 2], F32)

    with tile.TileContext(nc) as tc:
        with (
            tc.tile_pool(name="const", bufs=1) as const,
            tc.tile_pool(name="work", bufs=2) as work,
            tc.tile_pool(name="psum", bufs=2, space="PSUM") as psum,
            tc.tile_pool(name="psmall", bufs=1, space="PSUM") as psmall,
        ):
            wp, db128 = _emit_weight_prep(nc, const, psmall, w_row, w_col,
                                          b_row, b_col)
            _, offs = _emit_offsets(nc, const, work, idx, qmul=32, bmul=4096)

            fb = const.tile([P, 2 * N], BF16, tag="fb")
            nc.gpsimd.dma_start(
                out=fb[:].rearrange("p (kt n) -> p kt n", kt=2),
                in_=feats[:].rearrange("(kt p) n -> p kt n", p=P),
            )
            pt = psum.tile([P, 64], F32, tag="pt")
            for blk in range(16):
                nc.tensor.matmul(
                    pt[:, blk * 4:(blk + 1) * 4],
                    lhsT=fb[:, blk * P:(blk + 1) * P],
                    rhs=wp[:, 0:4], start=True, stop=False,
                )
                nc.tensor.matmul(
                    pt[:, blk * 4:(blk + 1) * 4],
                    lhsT=fb[:, N + blk * P:N + (blk + 1) * P],
                    rhs=wp[:, 4:8], start=False, stop=True,
                )
            sb_chunk = const.tile([P, 64], F32, tag="sb_chunk")
            nc.vector.tensor_copy(out=sb_chunk[:], in_=pt[:])
            chunk_write = nc.sync.dma_start(
                out=chunk_dram[:].rearrange("(q r) t -> q r t", q=P),
                in_=sb_chunk[:].rearrange("p (r t) -> p r t", t=2),
            )

            cc = nc.gpsimd.collective_compute(
                "AllGather",
                mybir.AluOpType.bypass,
                replica_groups=[list(range(NCORES))],
                ins=[chunk_dram[:]],
                outs=[t_dram[:]],
            )
            tile.add_dep_helper(cc.ins, chunk_write.ins, sync=True,
                                reason="allgather reads own chunk")

            _emit_gather_and_loss(nc, const, work, psmall, t_dram, offs,
                                  db128, partial, dep=cc)
    return nc


SHARDED = os.environ.get("KERNEL_VARIANT", "sharded") == "sharded"
_NC_CACHE = {}


def _get_nc(sharded=None):
    if sharded is None:
        sharded = SHARDED
    if sharded not in _NC_CACHE:
        nc = _build_nc_sharded() if sharded else _build_nc_replicated()
        nc.finalize()  # Bacc: regalloc, event sems, ACT table loads
        _NC_CACHE[sharded] = nc
    return _NC_CACHE[sharded]


def _pack_core_inputs(inputs, core, sharded):
    lists = [
        inputs["row_pos_b"], inputs["row_pos_i"], inputs["row_pos_j"],
        inputs["row_neg_b"], inputs["row_neg_i"],
        inputs["col_pos_b"], inputs["col_pos_i"], inputs["col_pos_j"],
        inputs["col_neg_b"], inputs["col_neg_i"],
    ]
    sl = slice(core * PAIRS, (core + 1) * PAIRS)
    arr = np.zeros((P, 10 * KW), np.int32)
    for j, lst in enumerate(lists):
        block = np.zeros(PADPAIRS, np.int32)
        block[:PAIRS] = np.asarray(lst[sl], np.int32)
        arr[:, j * KW:(j + 1) * KW] = block.reshape(P, KW)
    feats = np.asarray(inputs["all_features"], np.float32)
    im = {
        "w_row": np.ascontiguousarray(np.asarray(inputs["W_row"], np.float32)),
        "w_col": np.ascontiguousarray(np.asarray(inputs["W_col"], np.float32)),
        "b_row": np.ascontiguousarray(
            np.asarray(inputs["b_row"], np.float32).reshape(1, 2)),
        "b_col": np.ascontiguousarray(
            np.asarray(inputs["b_col"], np.float32).reshape(1, 2)),
        "idx": arr,
    }
    if sharded:
        im["feats_my"] = np.ascontiguousarray(feats[core])
    else:
        im["feats"] = np.ascontiguousarray(feats)
    return im


def run(inputs, trace=False, sharded=None):
    if sharded is None:
        sharded = SHARDED
    nc = _get_nc(sharded)
    in_maps = [_pack_core_inputs(inputs, c, sharded) for c in range(NCORES)]
    res = run_bass_kernel_spmd(nc, in_maps, list(range(NCORES)), trace=trace)
    partials = np.array(
        [res.results[c]["partial"][0] for c in range(NCORES)], np.float32
    )
    out = np.array([partials.sum()], np.float32)
    return out, res


def kernel(**inputs):
    out, _ = run(inputs, trace=False)
    return out
